# revision 18
# baseline (speedup 1.0000x reference)
"""DGCNN kernel for 8 Trainium2 NeuronCores (data-parallel over batch).

Pipeline (per core, batch shard of 256):
  host:   build normalized adjacency A, A2=A@A; compute BN mean/var on host
          and fold the BN scale into the lin weights and the BN bias + lin
          bias into fc0's bias; fold A2 into fc0 weights; pre-transpose x to
          [F, (node, batch)] bf16 so the device needs no transposes or
          stats pass.
          Host also applies the (BN-scaled) 512->128 lin projection, so the
          device input per core is u2 [128, 15872] bf16 (4 MB instead of
          16 MB of x), computed with f32 BLAS.
  device: a single dense matmul stream: fc0 (7936->3968, A2-folded
          weights) then fc1/fc2/fc3, each with fused ReLU+bias PSUM
          eviction.  Output [2, 256] per core; host glues.
"""

import numpy as np
import ml_dtypes

_B, _N, _F, _H, _C = 2048, 62, 512, 128, 2
_NCORES = 8
_BC = _B // _NCORES          # 256 samples per core
_T = _N * _BC                # 15872 tokens per core (node-major)
_D1, _D2, _D3 = 3968, 2048, 1024   # fc output dims (fc1/fc2 zero-padded)
_EPS_BN = 1e-5
_EPS_NORM = 1e-10

_COMPILED = None


def _normalized_adj(edge_weight):
    xs, ys = np.tril_indices(_N)
    Wm = np.zeros((_N, _N), np.float32)
    Wm[xs, ys] = edge_weight
    Wm = Wm + Wm.T - np.diag(np.diag(Wm))
    A = np.maximum(Wm, np.float32(0.0))
    dinv = (1.0 / np.sqrt(A.sum(1) + np.float32(_EPS_NORM))).astype(np.float32)
    A = dinv[:, None] * A * dinv[None, :]
    deg = A.sum(1)
    dis = np.where(deg > 0, deg ** -0.5, 0.0).astype(np.float32)
    return (dis[:, None] * A * dis[None, :]).astype(np.float32)


def _host_prep(inputs):
    f = lambda k: np.ascontiguousarray(np.asarray(inputs[k]), dtype=np.float32)
    x = f("x")
    edge_weight = f("edge_weight")
    gamma, beta = f("bn_gamma"), f("bn_beta")
    lin_w, lin_b = f("lin_w"), f("lin_b")
    fc0_w, fc0_b = f("fc0_w"), f("fc0_b")
    fc1_w, fc1_b = f("fc1_w"), f("fc1_b")
    fc2_w, fc2_b = f("fc2_w"), f("fc2_b")
    fc3_w, fc3_b = f("fc3_w"), f("fc3_b")

    A = _normalized_adj(edge_weight)
    A2 = (A @ A).astype(np.float32)
    r = A2.sum(1).astype(np.float32)                      # [N]

    # BatchNorm affine params from full-batch stats (train-mode BN)
    xf = x.reshape(-1, _F)
    mean = xf.mean(0, dtype=np.float64)
    var = np.square(xf, dtype=np.float64).mean(0) - mean * mean
    a = (gamma / np.sqrt(var + _EPS_BN)).astype(np.float32)     # scale
    c = (beta - mean.astype(np.float32) * a).astype(np.float32)  # bias

    W0r = fc0_w.reshape(_D1, _N, _H)                      # [o, i, h]
    # fold the 2-hop propagation into fc0:  W0p[o,j,h] = sum_i W0r[o,i,h] A2[i,j]
    W0p = np.matmul(W0r.transpose(0, 2, 1), A2).transpose(0, 2, 1)
    W0p = np.ascontiguousarray(W0p, dtype=np.float32)     # [o, j, h]

    # lhsT tile layouts (partition dim = contraction-within-tile)
    t = W0p.reshape(31, 128, 2, 31, 128)                  # [m, oi, half, kl, h]
    w0 = np.ascontiguousarray(t.transpose(0, 4, 2, 3, 1)) # [31, 128, 2, 31*128->]
    w0 = w0.reshape(31, 128, 2, 3968)

    w1p = np.zeros((_D2, _D1), np.float32)
    w1p[: fc1_w.shape[0]] = fc1_w
    w1 = np.ascontiguousarray(
        w1p.reshape(16, 128, 31, 128).transpose(0, 3, 2, 1)
    ).reshape(16, 128, 3968)

    w2p = np.zeros((_D3, _D2), np.float32)
    w2p[: fc2_w.shape[0], : fc2_w.shape[1]] = fc2_w
    w2 = np.ascontiguousarray(
        w2p.reshape(8, 128, 16, 128).transpose(0, 3, 2, 1)
    ).reshape(8, 128, 2048)

    w3p = np.zeros((_C, _D3), np.float32)
    w3p[:, : fc3_w.shape[1]] = fc3_w
    w3 = np.ascontiguousarray(
        w3p.reshape(_C, 8, 128).transpose(2, 1, 0)
    ).reshape(128, 16)

    waf = lin_w.T * a[:, None]                            # [F, H] BN-folded
    # fc0 bias: fc0_b + W0r.lin_b + P1.(lin_w @ c)  (all BN/lin bias paths)
    P1 = np.einsum("oih,i->oh", W0r, r).astype(np.float32)      # [o, h]
    q = np.einsum("oih,h->o", W0r, lin_b).astype(np.float32)
    v = P1 @ (lin_w @ c)
    b0 = np.ascontiguousarray((fc0_b + q + v).reshape(31, 128).T)  # [128, 31]
    b1p = np.zeros((_D2,), np.float32); b1p[: fc1_b.shape[0]] = fc1_b
    b1 = np.ascontiguousarray(b1p.reshape(16, 128).T)
    b2p = np.zeros((_D3,), np.float32); b2p[: fc2_b.shape[0]] = fc2_b
    b2 = np.ascontiguousarray(b2p.reshape(8, 128).T)
    b3 = np.ascontiguousarray(fc3_b.reshape(_C, 1))

    bfc = lambda arr: np.ascontiguousarray(arr.astype(ml_dtypes.bfloat16))
    w0, w1, w2, w3 = bfc(w0), bfc(w1), bfc(w2), bfc(w3)
    shared = dict(w0=w0, w1=w1, w2=w2, w3=w3,
                  b0=b0, b1=b1, b2=b2, b3=b3)

    # host lin: u2[h, (j, b)] per core, node-major token order, bf16
    xp = x.transpose(1, 0, 2)                             # [N, B, F]
    in_maps = []
    for cix in range(_NCORES):
        xs = np.ascontiguousarray(
            xp[:, cix * _BC:(cix + 1) * _BC, :]).reshape(_T, _F)
        u2c = np.ascontiguousarray((xs @ waf).T)          # [H, T] f32
        in_maps.append(dict(shared, u2=bfc(u2c)))
    return in_maps


def _build_nc():
    from contextlib import ExitStack
    import concourse.bacc as bacc
    import concourse.tile as tile
    import concourse.mybir as mybir
    from concourse.bass import ts

    dt = mybir.dt
    f32, bf16 = dt.float32, dt.bfloat16
    AF = mybir.ActivationFunctionType

    nc = bacc.Bacc("TRN2", target_bir_lowering=False, debug=False)

    u2d = nc.dram_tensor("u2", [128, _T], bf16, kind="ExternalInput").ap()
    w0 = nc.dram_tensor("w0", [31, 128, 2, 3968], bf16, kind="ExternalInput").ap()
    w1 = nc.dram_tensor("w1", [16, 128, 3968], bf16, kind="ExternalInput").ap()
    w2 = nc.dram_tensor("w2", [8, 128, 2048], bf16, kind="ExternalInput").ap()
    w3 = nc.dram_tensor("w3", [128, 16], bf16, kind="ExternalInput").ap()
    b0 = nc.dram_tensor("b0", [128, 31], f32, kind="ExternalInput").ap()
    b1 = nc.dram_tensor("b1", [128, 16], f32, kind="ExternalInput").ap()
    b2 = nc.dram_tensor("b2", [128, 8], f32, kind="ExternalInput").ap()
    b3 = nc.dram_tensor("b3", [_C, 1], f32, kind="ExternalInput").ap()
    outd = nc.dram_tensor("out", [_C, _BC], f32, kind="ExternalOutput").ap()

    with tile.TileContext(nc) as tc, ExitStack() as ctx:
        cpool = ctx.enter_context(tc.tile_pool(name="const", bufs=1))
        wpool = ctx.enter_context(tc.tile_pool(name="w", bufs=6))
        upool = ctx.enter_context(tc.tile_pool(name="u", bufs=1))
        hpool = ctx.enter_context(tc.tile_pool(name="h", bufs=1))
        spool = ctx.enter_context(tc.tile_pool(name="small", bufs=1))
        fpsum = ctx.enter_context(tc.tile_pool(name="fps", bufs=2, space="PSUM"))
        wpsum = ctx.enter_context(tc.tile_pool(name="wps", bufs=1, space="PSUM"))

        # ---- constants (scalar HWDGE queue) ----
        b0s = cpool.tile([128, 31], f32, tag="b0s")
        nc.scalar.dma_start(b0s[:], b0)
        b1s = cpool.tile([128, 16], f32, tag="b1s")
        nc.scalar.dma_start(b1s[:], b1)
        b2s = cpool.tile([128, 8], f32, tag="b2s")
        nc.scalar.dma_start(b2s[:], b2)
        b3s = cpool.tile([_C, 1], f32, tag="b3s")
        nc.scalar.dma_start(b3s[:], b3)
        w3s = cpool.tile([128, 16], bf16, tag="w3s")
        nc.scalar.dma_start(w3s[:], w3)

        # ---- PE warmup: keep HAM busy until the first fc0 operands land ----
        wps = wpsum.tile([16, 16], f32, tag="warm")
        for _ in range(128):
            nc.tensor.matmul(wps[:], w3s[:], w3s[:], start=True, stop=True)

        # ---- u2 load (host-computed lin output), node-aligned chunks,
        # interleaved with fc0 m=0 weight half-strips on one FIFO queue so
        # the m=0 k-loop can start as soon as chunk 0 + half-strip 0 land.
        CW = 8 * 256                                      # 8 nodes per chunk
        widths = [CW] * 7 + [_T - 7 * CW]                 # 62 = 7*8 + 6 nodes
        st0a = wpool.tile([128, 3968], bf16, tag="w")
        nc.gpsimd.dma_start(st0a[:], w0[0, :, 0, :])
        u2t = []
        for ch, w in enumerate(widths):
            t = upool.tile([128, w], bf16, tag=f"u2_{ch}")
            nc.gpsimd.dma_start(t[:], u2d[:, ch * CW:ch * CW + w])
            u2t.append(t)
            if ch == 3:
                st0b = wpool.tile([128, 3968], bf16, tag="w")
                nc.gpsimd.dma_start(st0b[:], w0[0, :, 1, :])
        st0 = [st0a, st0b]

        def u2_ap(kk):
            # node kk's 256 batch columns inside the chunked u2 tiles
            pos = kk * 256
            ch, off = pos // CW, pos % CW
            return u2t[ch][:, off:off + 256]

        # ---- fc0 (62 k-tiles via two half-strips) ----
        h1 = hpool.tile([128, 31 * 256], bf16, tag="h1")
        for m in range(31):
            fp = fpsum.tile([128, 256], f32, tag="fp")
            for half in range(2):
                if m == 0:
                    st = st0[half]
                else:
                    st = wpool.tile([128, 3968], bf16, tag="w")
                    nc.gpsimd.dma_start(st[:], w0[m, :, half, :])
                for k in range(31):
                    kk = half * 31 + k
                    nc.tensor.matmul(fp[:], st[:, ts(k, 128)],
                                     u2_ap(kk),
                                     start=(kk == 0), stop=(kk == 61))
            nc.scalar.activation(h1[:, ts(m, 256)], fp[:], AF.Relu,
                                 bias=b0s[:, m:m + 1])

        # ---- fc1 ----
        h2 = hpool.tile([128, 16 * 256], bf16, tag="h2")
        for m in range(16):
            fp = fpsum.tile([128, 256], f32, tag="fp")
            st = wpool.tile([128, 3968], bf16, tag="w")
            nc.gpsimd.dma_start(st[:], w1[m, :, :])
            for k in range(31):
                nc.tensor.matmul(fp[:], st[:, ts(k, 128)],
                                 h1[:, ts(k, 256)],
                                 start=(k == 0), stop=(k == 30))
            nc.scalar.activation(h2[:, ts(m, 256)], fp[:], AF.Relu,
                                 bias=b1s[:, m:m + 1])

        # ---- fc2 ----
        h3 = hpool.tile([128, 8 * 256], bf16, tag="h3")
        for m in range(8):
            fp = fpsum.tile([128, 256], f32, tag="fp")
            st = wpool.tile([128, 2048], bf16, tag="w")
            nc.gpsimd.dma_start(st[:], w2[m, :, :])
            for k in range(16):
                nc.tensor.matmul(fp[:], st[:, ts(k, 128)],
                                 h2[:, ts(k, 256)],
                                 start=(k == 0), stop=(k == 15))
            nc.scalar.activation(h3[:, ts(m, 256)], fp[:], AF.Relu,
                                 bias=b2s[:, m:m + 1])

        # ---- fc3 ----
        fp3 = fpsum.tile([_C, 256], f32, tag="fp")
        for k in range(8):
            nc.tensor.matmul(fp3[:], w3s[:, ts(k, 2)],
                             h3[:, ts(k, 256)],
                             start=(k == 0), stop=(k == 7))
        osb = spool.tile([_C, 256], f32, tag="osb")
        nc.scalar.activation(osb[:], fp3[:], AF.Identity, bias=b3s[:])
        nc.sync.dma_start(outd, osb[:])

    nc.compile()
    return nc


def kernel(**inputs):
    global _COMPILED
    from concourse.bass_utils import run_bass_kernel_spmd

    in_maps = _host_prep(inputs)
    if _COMPILED is None:
        _COMPILED = _build_nc()
    res = run_bass_kernel_spmd(_COMPILED, in_maps,
                               core_ids=list(range(_NCORES)))
    out = np.concatenate([res.results[c]["out"].T for c in range(_NCORES)],
                         axis=0)
    return np.ascontiguousarray(out, dtype=np.float32)


# revision 19
# speedup vs baseline: 1.0072x; 1.0072x over previous
"""DGCNN kernel for 8 Trainium2 NeuronCores (data-parallel over batch).

Pipeline (per core, batch shard of 256):
  host:   build normalized adjacency A, A2=A@A; compute BN mean/var on host
          and fold the BN scale into the lin weights and the BN bias + lin
          bias into fc0's bias; fold A2 into fc0 weights; pre-transpose x to
          [F, (node, batch)] bf16 so the device needs no transposes or
          stats pass.
          Host also applies the (BN-scaled) 512->128 lin projection, so the
          device input per core is u2 [128, 15872] bf16 (4 MB instead of
          16 MB of x), computed with f32 BLAS.
  device: a single dense matmul stream: fc0 (7936->3968, A2-folded
          weights) then fc1/fc2/fc3, each with fused ReLU+bias PSUM
          eviction.  Output [2, 256] per core; host glues.
"""

import numpy as np
import ml_dtypes

_B, _N, _F, _H, _C = 2048, 62, 512, 128, 2
_NCORES = 8
_BC = _B // _NCORES          # 256 samples per core
_T = _N * _BC                # 15872 tokens per core (node-major)
_D1, _D2, _D3 = 3968, 2048, 1024   # fc output dims (fc1/fc2 zero-padded)
_EPS_BN = 1e-5
_EPS_NORM = 1e-10

_COMPILED = None


def _normalized_adj(edge_weight):
    xs, ys = np.tril_indices(_N)
    Wm = np.zeros((_N, _N), np.float32)
    Wm[xs, ys] = edge_weight
    Wm = Wm + Wm.T - np.diag(np.diag(Wm))
    A = np.maximum(Wm, np.float32(0.0))
    dinv = (1.0 / np.sqrt(A.sum(1) + np.float32(_EPS_NORM))).astype(np.float32)
    A = dinv[:, None] * A * dinv[None, :]
    deg = A.sum(1)
    dis = np.where(deg > 0, deg ** -0.5, 0.0).astype(np.float32)
    return (dis[:, None] * A * dis[None, :]).astype(np.float32)


def _host_prep(inputs):
    f = lambda k: np.ascontiguousarray(np.asarray(inputs[k]), dtype=np.float32)
    x = f("x")
    edge_weight = f("edge_weight")
    gamma, beta = f("bn_gamma"), f("bn_beta")
    lin_w, lin_b = f("lin_w"), f("lin_b")
    fc0_w, fc0_b = f("fc0_w"), f("fc0_b")
    fc1_w, fc1_b = f("fc1_w"), f("fc1_b")
    fc2_w, fc2_b = f("fc2_w"), f("fc2_b")
    fc3_w, fc3_b = f("fc3_w"), f("fc3_b")

    A = _normalized_adj(edge_weight)
    A2 = (A @ A).astype(np.float32)
    r = A2.sum(1).astype(np.float32)                      # [N]

    # BatchNorm affine params from full-batch stats (train-mode BN)
    xf = x.reshape(-1, _F)
    mean = xf.mean(0, dtype=np.float64)
    var = np.square(xf, dtype=np.float64).mean(0) - mean * mean
    a = (gamma / np.sqrt(var + _EPS_BN)).astype(np.float32)     # scale
    c = (beta - mean.astype(np.float32) * a).astype(np.float32)  # bias

    W0r = fc0_w.reshape(_D1, _N, _H)                      # [o, i, h]
    # fold the 2-hop propagation into fc0:  W0p[o,j,h] = sum_i W0r[o,i,h] A2[i,j]
    W0p = np.matmul(W0r.transpose(0, 2, 1), A2).transpose(0, 2, 1)
    W0p = np.ascontiguousarray(W0p, dtype=np.float32)     # [o, j, h]

    # lhsT tile layouts (partition dim = contraction-within-tile)
    t = W0p.reshape(31, 128, 2, 31, 128)                  # [m, oi, half, kl, h]
    w0 = np.ascontiguousarray(t.transpose(0, 4, 2, 3, 1)) # [31, 128, 2, 31*128->]
    w0 = w0.reshape(31, 128, 2, 3968)

    w1p = np.zeros((_D2, _D1), np.float32)
    w1p[: fc1_w.shape[0]] = fc1_w
    w1 = np.ascontiguousarray(
        w1p.reshape(16, 128, 31, 128).transpose(0, 3, 2, 1)
    ).reshape(16, 128, 3968)

    w2p = np.zeros((_D3, _D2), np.float32)
    w2p[: fc2_w.shape[0], : fc2_w.shape[1]] = fc2_w
    w2 = np.ascontiguousarray(
        w2p.reshape(8, 128, 16, 128).transpose(0, 3, 2, 1)
    ).reshape(8, 128, 2048)

    w3p = np.zeros((_C, _D3), np.float32)
    w3p[:, : fc3_w.shape[1]] = fc3_w
    w3 = np.ascontiguousarray(
        w3p.reshape(_C, 8, 128).transpose(2, 1, 0)
    ).reshape(128, 16)

    waf = lin_w.T * a[:, None]                            # [F, H] BN-folded
    # fc0 bias: fc0_b + W0r.lin_b + P1.(lin_w @ c)  (all BN/lin bias paths)
    P1 = np.einsum("oih,i->oh", W0r, r).astype(np.float32)      # [o, h]
    q = np.einsum("oih,h->o", W0r, lin_b).astype(np.float32)
    v = P1 @ (lin_w @ c)
    b0 = np.ascontiguousarray((fc0_b + q + v).reshape(31, 128).T)  # [128, 31]
    b1p = np.zeros((_D2,), np.float32); b1p[: fc1_b.shape[0]] = fc1_b
    b1 = np.ascontiguousarray(b1p.reshape(16, 128).T)
    b2p = np.zeros((_D3,), np.float32); b2p[: fc2_b.shape[0]] = fc2_b
    b2 = np.ascontiguousarray(b2p.reshape(8, 128).T)
    b3 = np.ascontiguousarray(fc3_b.reshape(_C, 1))

    bfc = lambda arr: np.ascontiguousarray(arr.astype(ml_dtypes.bfloat16))
    w0, w1, w2, w3 = bfc(w0), bfc(w1), bfc(w2), bfc(w3)
    shared = dict(w0=w0, w1=w1, w2=w2, w3=w3,
                  b0=b0, b1=b1, b2=b2, b3=b3)

    # host lin: u2[h, (j, b)] per core, node-major token order, bf16
    xp = x.transpose(1, 0, 2)                             # [N, B, F]
    in_maps = []
    for cix in range(_NCORES):
        xs = np.ascontiguousarray(
            xp[:, cix * _BC:(cix + 1) * _BC, :]).reshape(_T, _F)
        u2c = np.ascontiguousarray((xs @ waf).T)          # [H, T] f32
        in_maps.append(dict(shared, u2=bfc(u2c)))
    return in_maps


def _build_nc():
    from contextlib import ExitStack
    import concourse.bacc as bacc
    import concourse.tile as tile
    import concourse.mybir as mybir
    from concourse.bass import ts

    dt = mybir.dt
    f32, bf16 = dt.float32, dt.bfloat16
    AF = mybir.ActivationFunctionType

    nc = bacc.Bacc("TRN2", target_bir_lowering=False, debug=False)

    u2d = nc.dram_tensor("u2", [128, _T], bf16, kind="ExternalInput").ap()
    w0 = nc.dram_tensor("w0", [31, 128, 2, 3968], bf16, kind="ExternalInput").ap()
    w1 = nc.dram_tensor("w1", [16, 128, 3968], bf16, kind="ExternalInput").ap()
    w2 = nc.dram_tensor("w2", [8, 128, 2048], bf16, kind="ExternalInput").ap()
    w3 = nc.dram_tensor("w3", [128, 16], bf16, kind="ExternalInput").ap()
    b0 = nc.dram_tensor("b0", [128, 31], f32, kind="ExternalInput").ap()
    b1 = nc.dram_tensor("b1", [128, 16], f32, kind="ExternalInput").ap()
    b2 = nc.dram_tensor("b2", [128, 8], f32, kind="ExternalInput").ap()
    b3 = nc.dram_tensor("b3", [_C, 1], f32, kind="ExternalInput").ap()
    outd = nc.dram_tensor("out", [_C, _BC], f32, kind="ExternalOutput").ap()

    with tile.TileContext(nc) as tc, ExitStack() as ctx:
        cpool = ctx.enter_context(tc.tile_pool(name="const", bufs=1))
        wpool = ctx.enter_context(tc.tile_pool(name="w", bufs=6))
        upool = ctx.enter_context(tc.tile_pool(name="u", bufs=1))
        hpool = ctx.enter_context(tc.tile_pool(name="h", bufs=1))
        spool = ctx.enter_context(tc.tile_pool(name="small", bufs=1))
        fpsum = ctx.enter_context(tc.tile_pool(name="fps", bufs=2, space="PSUM"))
        wpsum = ctx.enter_context(tc.tile_pool(name="wps", bufs=1, space="PSUM"))

        # ---- constants (scalar HWDGE queue) ----
        b0s = cpool.tile([128, 31], f32, tag="b0s")
        nc.scalar.dma_start(b0s[:], b0)
        b1s = cpool.tile([128, 16], f32, tag="b1s")
        nc.scalar.dma_start(b1s[:], b1)
        b2s = cpool.tile([128, 8], f32, tag="b2s")
        nc.scalar.dma_start(b2s[:], b2)
        b3s = cpool.tile([_C, 1], f32, tag="b3s")
        nc.scalar.dma_start(b3s[:], b3)
        w3s = cpool.tile([128, 16], bf16, tag="w3s")
        nc.scalar.dma_start(w3s[:], w3)

        # ---- PE warmup: keep HAM busy until the first fc0 operands land ----
        wps = wpsum.tile([16, 16], f32, tag="warm")
        for _ in range(80):
            nc.tensor.matmul(wps[:], w3s[:], w3s[:], start=True, stop=True)

        # ---- u2 load (host-computed lin output), node-aligned chunks,
        # interleaved with fc0 m=0 weight half-strips on one FIFO queue so
        # the m=0 k-loop can start as soon as chunk 0 + half-strip 0 land.
        CW = 8 * 256                                      # 8 nodes per chunk
        widths = [CW] * 7 + [_T - 7 * CW]                 # 62 = 7*8 + 6 nodes
        st0a = wpool.tile([128, 3968], bf16, tag="w")
        nc.gpsimd.dma_start(st0a[:], w0[0, :, 0, :])
        u2t = []
        for ch, w in enumerate(widths):
            t = upool.tile([128, w], bf16, tag=f"u2_{ch}")
            nc.gpsimd.dma_start(t[:], u2d[:, ch * CW:ch * CW + w])
            u2t.append(t)
            if ch == 3:
                st0b = wpool.tile([128, 3968], bf16, tag="w")
                nc.gpsimd.dma_start(st0b[:], w0[0, :, 1, :])
        st0 = [st0a, st0b]

        def u2_ap(kk):
            # node kk's 256 batch columns inside the chunked u2 tiles
            pos = kk * 256
            ch, off = pos // CW, pos % CW
            return u2t[ch][:, off:off + 256]

        # ---- fc0 (62 k-tiles via two half-strips) ----
        h1 = hpool.tile([128, 31 * 256], bf16, tag="h1")
        for m in range(31):
            fp = fpsum.tile([128, 256], f32, tag="fp")
            for half in range(2):
                if m == 0:
                    st = st0[half]
                else:
                    st = wpool.tile([128, 3968], bf16, tag="w")
                    nc.gpsimd.dma_start(st[:], w0[m, :, half, :])
                for k in range(31):
                    kk = half * 31 + k
                    nc.tensor.matmul(fp[:], st[:, ts(k, 128)],
                                     u2_ap(kk),
                                     start=(kk == 0), stop=(kk == 61))
            nc.scalar.activation(h1[:, ts(m, 256)], fp[:], AF.Relu,
                                 bias=b0s[:, m:m + 1])

        # ---- fc1 ----
        h2 = hpool.tile([128, 16 * 256], bf16, tag="h2")
        for m in range(16):
            fp = fpsum.tile([128, 256], f32, tag="fp")
            st = wpool.tile([128, 3968], bf16, tag="w")
            nc.gpsimd.dma_start(st[:], w1[m, :, :])
            for k in range(31):
                nc.tensor.matmul(fp[:], st[:, ts(k, 128)],
                                 h1[:, ts(k, 256)],
                                 start=(k == 0), stop=(k == 30))
            nc.scalar.activation(h2[:, ts(m, 256)], fp[:], AF.Relu,
                                 bias=b1s[:, m:m + 1])

        # ---- fc2 ----
        h3 = hpool.tile([128, 8 * 256], bf16, tag="h3")
        for m in range(8):
            fp = fpsum.tile([128, 256], f32, tag="fp")
            st = wpool.tile([128, 2048], bf16, tag="w")
            nc.gpsimd.dma_start(st[:], w2[m, :, :])
            for k in range(16):
                nc.tensor.matmul(fp[:], st[:, ts(k, 128)],
                                 h2[:, ts(k, 256)],
                                 start=(k == 0), stop=(k == 15))
            nc.scalar.activation(h3[:, ts(m, 256)], fp[:], AF.Relu,
                                 bias=b2s[:, m:m + 1])

        # ---- fc3 ----
        fp3 = fpsum.tile([_C, 256], f32, tag="fp")
        for k in range(8):
            nc.tensor.matmul(fp3[:], w3s[:, ts(k, 2)],
                             h3[:, ts(k, 256)],
                             start=(k == 0), stop=(k == 7))
        osb = spool.tile([_C, 256], f32, tag="osb")
        nc.scalar.activation(osb[:], fp3[:], AF.Identity, bias=b3s[:])
        nc.sync.dma_start(outd, osb[:])

    nc.compile()
    return nc


def kernel(**inputs):
    global _COMPILED
    from concourse.bass_utils import run_bass_kernel_spmd

    in_maps = _host_prep(inputs)
    if _COMPILED is None:
        _COMPILED = _build_nc()
    res = run_bass_kernel_spmd(_COMPILED, in_maps,
                               core_ids=list(range(_NCORES)))
    out = np.concatenate([res.results[c]["out"].T for c in range(_NCORES)],
                         axis=0)
    return np.ascontiguousarray(out, dtype=np.float32)


# revision 20
# speedup vs baseline: 1.0103x; 1.0031x over previous
"""DGCNN kernel for 8 Trainium2 NeuronCores (data-parallel over batch).

Pipeline (per core, batch shard of 256):
  host:   build normalized adjacency A, A2=A@A; compute BN mean/var on host
          and fold the BN scale into the lin weights and the BN bias + lin
          bias into fc0's bias; fold A2 into fc0 weights; pre-transpose x to
          [F, (node, batch)] bf16 so the device needs no transposes or
          stats pass.
          Host also applies the (BN-scaled) 512->128 lin projection, so the
          device input per core is u2 [128, 15872] bf16 (4 MB instead of
          16 MB of x), computed with f32 BLAS.
  device: a single dense matmul stream: fc0 (7936->3968, A2-folded
          weights) then fc1/fc2/fc3, each with fused ReLU+bias PSUM
          eviction.  Output [2, 256] per core; host glues.
"""

import numpy as np
import ml_dtypes

_B, _N, _F, _H, _C = 2048, 62, 512, 128, 2
_NCORES = 8
_BC = _B // _NCORES          # 256 samples per core
_T = _N * _BC                # 15872 tokens per core (node-major)
_D1, _D2, _D3 = 3968, 2048, 1024   # fc output dims (fc1/fc2 zero-padded)
_EPS_BN = 1e-5
_EPS_NORM = 1e-10

_COMPILED = None


def _normalized_adj(edge_weight):
    xs, ys = np.tril_indices(_N)
    Wm = np.zeros((_N, _N), np.float32)
    Wm[xs, ys] = edge_weight
    Wm = Wm + Wm.T - np.diag(np.diag(Wm))
    A = np.maximum(Wm, np.float32(0.0))
    dinv = (1.0 / np.sqrt(A.sum(1) + np.float32(_EPS_NORM))).astype(np.float32)
    A = dinv[:, None] * A * dinv[None, :]
    deg = A.sum(1)
    dis = np.where(deg > 0, deg ** -0.5, 0.0).astype(np.float32)
    return (dis[:, None] * A * dis[None, :]).astype(np.float32)


def _host_prep(inputs):
    f = lambda k: np.ascontiguousarray(np.asarray(inputs[k]), dtype=np.float32)
    x = f("x")
    edge_weight = f("edge_weight")
    gamma, beta = f("bn_gamma"), f("bn_beta")
    lin_w, lin_b = f("lin_w"), f("lin_b")
    fc0_w, fc0_b = f("fc0_w"), f("fc0_b")
    fc1_w, fc1_b = f("fc1_w"), f("fc1_b")
    fc2_w, fc2_b = f("fc2_w"), f("fc2_b")
    fc3_w, fc3_b = f("fc3_w"), f("fc3_b")

    A = _normalized_adj(edge_weight)
    A2 = (A @ A).astype(np.float32)
    r = A2.sum(1).astype(np.float32)                      # [N]

    # BatchNorm affine params from full-batch stats (train-mode BN)
    xf = x.reshape(-1, _F)
    mean = xf.mean(0, dtype=np.float64)
    var = np.square(xf, dtype=np.float64).mean(0) - mean * mean
    a = (gamma / np.sqrt(var + _EPS_BN)).astype(np.float32)     # scale
    c = (beta - mean.astype(np.float32) * a).astype(np.float32)  # bias

    W0r = fc0_w.reshape(_D1, _N, _H)                      # [o, i, h]
    # fold the 2-hop propagation into fc0:  W0p[o,j,h] = sum_i W0r[o,i,h] A2[i,j]
    W0p = np.matmul(W0r.transpose(0, 2, 1), A2).transpose(0, 2, 1)
    W0p = np.ascontiguousarray(W0p, dtype=np.float32)     # [o, j, h]

    # lhsT tile layouts (partition dim = contraction-within-tile)
    t = W0p.reshape(31, 128, 2, 31, 128)                  # [m, oi, half, kl, h]
    w0 = np.ascontiguousarray(t.transpose(0, 2, 4, 3, 1)) # [m, half, h, kl, oi]
    w0 = w0.reshape(31, 2, 128, 3968)

    w1p = np.zeros((_D2, _D1), np.float32)
    w1p[: fc1_w.shape[0]] = fc1_w
    w1 = np.ascontiguousarray(
        w1p.reshape(16, 128, 31, 128).transpose(0, 3, 2, 1)
    ).reshape(16, 128, 3968)

    w2p = np.zeros((_D3, _D2), np.float32)
    w2p[: fc2_w.shape[0], : fc2_w.shape[1]] = fc2_w
    w2 = np.ascontiguousarray(
        w2p.reshape(8, 128, 16, 128).transpose(0, 3, 2, 1)
    ).reshape(8, 128, 2048)

    w3p = np.zeros((_C, _D3), np.float32)
    w3p[:, : fc3_w.shape[1]] = fc3_w
    w3 = np.ascontiguousarray(
        w3p.reshape(_C, 8, 128).transpose(2, 1, 0)
    ).reshape(128, 16)

    waf = lin_w.T * a[:, None]                            # [F, H] BN-folded
    # fc0 bias: fc0_b + W0r.lin_b + P1.(lin_w @ c)  (all BN/lin bias paths)
    P1 = np.einsum("oih,i->oh", W0r, r).astype(np.float32)      # [o, h]
    q = np.einsum("oih,h->o", W0r, lin_b).astype(np.float32)
    v = P1 @ (lin_w @ c)
    b0 = np.ascontiguousarray((fc0_b + q + v).reshape(31, 128).T)  # [128, 31]
    b1p = np.zeros((_D2,), np.float32); b1p[: fc1_b.shape[0]] = fc1_b
    b1 = np.ascontiguousarray(b1p.reshape(16, 128).T)
    b2p = np.zeros((_D3,), np.float32); b2p[: fc2_b.shape[0]] = fc2_b
    b2 = np.ascontiguousarray(b2p.reshape(8, 128).T)
    b3 = np.ascontiguousarray(fc3_b.reshape(_C, 1))

    bfc = lambda arr: np.ascontiguousarray(arr.astype(ml_dtypes.bfloat16))
    w0, w1, w2, w3 = bfc(w0), bfc(w1), bfc(w2), bfc(w3)
    shared = dict(w0=w0, w1=w1, w2=w2, w3=w3,
                  b0=b0, b1=b1, b2=b2, b3=b3)

    # host lin: u2[h, (j, b)] per core, node-major token order, bf16
    xp = x.transpose(1, 0, 2)                             # [N, B, F]
    in_maps = []
    for cix in range(_NCORES):
        xs = np.ascontiguousarray(
            xp[:, cix * _BC:(cix + 1) * _BC, :]).reshape(_T, _F)
        u2c = np.ascontiguousarray((xs @ waf).T)          # [H, T] f32
        in_maps.append(dict(shared, u2=bfc(u2c)))
    return in_maps


def _build_nc():
    from contextlib import ExitStack
    import concourse.bacc as bacc
    import concourse.tile as tile
    import concourse.mybir as mybir
    from concourse.bass import ts

    dt = mybir.dt
    f32, bf16 = dt.float32, dt.bfloat16
    AF = mybir.ActivationFunctionType

    nc = bacc.Bacc("TRN2", target_bir_lowering=False, debug=False)

    u2d = nc.dram_tensor("u2", [128, _T], bf16, kind="ExternalInput").ap()
    w0 = nc.dram_tensor("w0", [31, 2, 128, 3968], bf16, kind="ExternalInput").ap()
    w1 = nc.dram_tensor("w1", [16, 128, 3968], bf16, kind="ExternalInput").ap()
    w2 = nc.dram_tensor("w2", [8, 128, 2048], bf16, kind="ExternalInput").ap()
    w3 = nc.dram_tensor("w3", [128, 16], bf16, kind="ExternalInput").ap()
    b0 = nc.dram_tensor("b0", [128, 31], f32, kind="ExternalInput").ap()
    b1 = nc.dram_tensor("b1", [128, 16], f32, kind="ExternalInput").ap()
    b2 = nc.dram_tensor("b2", [128, 8], f32, kind="ExternalInput").ap()
    b3 = nc.dram_tensor("b3", [_C, 1], f32, kind="ExternalInput").ap()
    outd = nc.dram_tensor("out", [_C, _BC], f32, kind="ExternalOutput").ap()

    with tile.TileContext(nc) as tc, ExitStack() as ctx:
        cpool = ctx.enter_context(tc.tile_pool(name="const", bufs=1))
        wpool = ctx.enter_context(tc.tile_pool(name="w", bufs=6))
        upool = ctx.enter_context(tc.tile_pool(name="u", bufs=1))
        hpool = ctx.enter_context(tc.tile_pool(name="h", bufs=1))
        spool = ctx.enter_context(tc.tile_pool(name="small", bufs=1))
        fpsum = ctx.enter_context(tc.tile_pool(name="fps", bufs=2, space="PSUM"))
        wpsum = ctx.enter_context(tc.tile_pool(name="wps", bufs=1, space="PSUM"))

        # ---- constants (scalar HWDGE queue) ----
        b0s = cpool.tile([128, 31], f32, tag="b0s")
        nc.scalar.dma_start(b0s[:], b0)
        b1s = cpool.tile([128, 16], f32, tag="b1s")
        nc.scalar.dma_start(b1s[:], b1)
        b2s = cpool.tile([128, 8], f32, tag="b2s")
        nc.scalar.dma_start(b2s[:], b2)
        b3s = cpool.tile([_C, 1], f32, tag="b3s")
        nc.scalar.dma_start(b3s[:], b3)
        w3s = cpool.tile([128, 16], bf16, tag="w3s")
        nc.scalar.dma_start(w3s[:], w3)

        # ---- PE warmup: keep HAM busy until the first fc0 operands land ----
        wps = wpsum.tile([16, 16], f32, tag="warm")
        for _ in range(80):
            nc.tensor.matmul(wps[:], w3s[:], w3s[:], start=True, stop=True)

        # ---- u2 load (host-computed lin output), node-aligned chunks,
        # interleaved with fc0 m=0 weight half-strips on one FIFO queue so
        # the m=0 k-loop can start as soon as chunk 0 + half-strip 0 land.
        CW = 8 * 256                                      # 8 nodes per chunk
        widths = [CW] * 7 + [_T - 7 * CW]                 # 62 = 7*8 + 6 nodes
        st0a = wpool.tile([128, 3968], bf16, tag="w")
        nc.gpsimd.dma_start(st0a[:], w0[0, 0])
        u2t = []
        for ch, w in enumerate(widths):
            t = upool.tile([128, w], bf16, tag=f"u2_{ch}")
            nc.gpsimd.dma_start(t[:], u2d[:, ch * CW:ch * CW + w])
            u2t.append(t)
            if ch == 3:
                st0b = wpool.tile([128, 3968], bf16, tag="w")
                nc.gpsimd.dma_start(st0b[:], w0[0, 1])
        st0 = [st0a, st0b]

        def u2_ap(kk):
            # node kk's 256 batch columns inside the chunked u2 tiles
            pos = kk * 256
            ch, off = pos // CW, pos % CW
            return u2t[ch][:, off:off + 256]

        # ---- fc0 (62 k-tiles via two half-strips) ----
        h1 = hpool.tile([128, 31 * 256], bf16, tag="h1")
        for m in range(31):
            fp = fpsum.tile([128, 256], f32, tag="fp")
            for half in range(2):
                if m == 0:
                    st = st0[half]
                else:
                    st = wpool.tile([128, 3968], bf16, tag="w")
                    nc.gpsimd.dma_start(st[:], w0[m, half])
                for k in range(31):
                    kk = half * 31 + k
                    nc.tensor.matmul(fp[:], st[:, ts(k, 128)],
                                     u2_ap(kk),
                                     start=(kk == 0), stop=(kk == 61))
            nc.scalar.activation(h1[:, ts(m, 256)], fp[:], AF.Relu,
                                 bias=b0s[:, m:m + 1])

        # ---- fc1 ----
        h2 = hpool.tile([128, 16 * 256], bf16, tag="h2")
        for m in range(16):
            fp = fpsum.tile([128, 256], f32, tag="fp")
            st = wpool.tile([128, 3968], bf16, tag="w")
            nc.gpsimd.dma_start(st[:], w1[m, :, :])
            for k in range(31):
                nc.tensor.matmul(fp[:], st[:, ts(k, 128)],
                                 h1[:, ts(k, 256)],
                                 start=(k == 0), stop=(k == 30))
            nc.scalar.activation(h2[:, ts(m, 256)], fp[:], AF.Relu,
                                 bias=b1s[:, m:m + 1])

        # ---- fc2 ----
        h3 = hpool.tile([128, 8 * 256], bf16, tag="h3")
        for m in range(8):
            fp = fpsum.tile([128, 256], f32, tag="fp")
            st = wpool.tile([128, 2048], bf16, tag="w")
            nc.gpsimd.dma_start(st[:], w2[m, :, :])
            for k in range(16):
                nc.tensor.matmul(fp[:], st[:, ts(k, 128)],
                                 h2[:, ts(k, 256)],
                                 start=(k == 0), stop=(k == 15))
            nc.scalar.activation(h3[:, ts(m, 256)], fp[:], AF.Relu,
                                 bias=b2s[:, m:m + 1])

        # ---- fc3 ----
        fp3 = fpsum.tile([_C, 256], f32, tag="fp")
        for k in range(8):
            nc.tensor.matmul(fp3[:], w3s[:, ts(k, 2)],
                             h3[:, ts(k, 256)],
                             start=(k == 0), stop=(k == 7))
        osb = spool.tile([_C, 256], f32, tag="osb")
        nc.scalar.activation(osb[:], fp3[:], AF.Identity, bias=b3s[:])
        nc.sync.dma_start(outd, osb[:])

    nc.compile()
    return nc


def kernel(**inputs):
    global _COMPILED
    from concourse.bass_utils import run_bass_kernel_spmd

    in_maps = _host_prep(inputs)
    if _COMPILED is None:
        _COMPILED = _build_nc()
    res = run_bass_kernel_spmd(_COMPILED, in_maps,
                               core_ids=list(range(_NCORES)))
    out = np.concatenate([res.results[c]["out"].T for c in range(_NCORES)],
                         axis=0)
    return np.ascontiguousarray(out, dtype=np.float32)
